# revision 2
# baseline (speedup 1.0000x reference)
"""LinkPredictor similarity kernel for 8 Trainium2 NeuronCores.

reference:
    sims = E @ E.T               # [16384, 16384], E = [16384, 512] fp32
    m, M = sims.min(), sims.max()
    sims = (sims - m) / (M - m + 1e-7)
    out  = sims[row_idx, col_idx]     # block-diag strict-upper-tri gather

Only the 128 diagonal [128,128] graph blocks are ever gathered, but the
global min needs every entry of sims. Two mathematical shortcuts:
  * sims is symmetric -> min over the block upper triangle suffices.
  * By Cauchy-Schwarz, s_ij <= |e_i||e_j| <= max_k |e_k|^2 = max diag,
    so the global max is exactly the max diagonal entry -- free from the
    bf16 diagonal blocks; the expensive sweep only tracks the min.

Distribution: 16 half-slabs of 1024 rows; core c owns half-slabs
{c, 15-c} and the 17 upper-triangle [1024,1024] blocks whose row
half-slab is one of those (every core gets exactly 17 blocks).

Per block the sweep runs fp8e4 (e4m3) matmuls in DoubleRow perf mode
(K=256 per instruction, 2x PE throughput): 2 matmuls accumulate each
[128,512] PSUM tile. A running elementwise min is kept two ways to
spread load: 3 of every 4 tiles go PSUM -> fp16 SBUF on ScalarE then
VectorE min; every 4th tile VectorE reads the fp32 PSUM directly.
Host-simulated numerics: fp8 min estimate is within ~0.3 of the exact
min (range spans ~790), total pipeline rel err ~2e-3 vs the 2e-2 gate.

The 16 diagonal graph blocks per core are computed in bf16 and written
out. Host combines mins, takes max from the block diagonals, normalizes
and gathers with the real row/col indices.
"""

import numpy as np
import ml_dtypes

N_GRAPHS = 128
G = 128
D = 512
N = N_GRAPHS * G          # 16384
EPS = 1e-7
NCORES = 8
HS = 1024                 # half-slab rows
NHS = N // HS             # 16 half-slabs
NBLK = 17                 # triangle blocks per core
KC = D // 128             # 4 contraction chunks of 128
MT = HS // 128            # 8 m-tiles per block
NT = HS // 512            # 2 n-tiles (512 wide) per block
GPC = 16                  # graphs per core

_CACHED = {}
LAST_RESULTS = None       # BassKernelResults of the most recent run


def _build_program():
    import concourse.bacc as bacc
    import concourse.mybir as mybir
    from concourse.tile import TileContext

    f32 = mybir.dt.float32
    f16 = mybir.dt.float16
    bf16 = mybir.dt.bfloat16
    f8 = mybir.dt.float8e4
    DR = mybir.MatmulPerfMode.DoubleRow

    nc = bacc.Bacc(target_bir_lowering=False)
    # per-partition-contiguous packing: [block, partition, d1, col]
    lhs = nc.declare_dram_parameter("lhs", [NBLK, 128, KC, HS], f8, isOutput=False)
    rhs = nc.declare_dram_parameter("rhs", [NBLK, 128, KC, HS], f8, isOutput=False)
    dg_in = nc.declare_dram_parameter("dg", [128, KC, GPC * G], bf16, isOutput=False)
    diag_out = nc.declare_dram_parameter("diag_out", [GPC, G, G], f32, isOutput=True)
    mins = nc.declare_dram_parameter("mins", [128, 2], f32, isOutput=True)

    with TileContext(nc) as tc:
        with (
            tc.tile_pool(name="stream", bufs=4) as stream,
            tc.tile_pool(name="small", bufs=4) as small,
            tc.tile_pool(name="acc", bufs=1) as accp,
            tc.tile_pool(name="ps", bufs=5, space="PSUM") as ps,
            tc.tile_pool(name="psd", bufs=2, space="PSUM") as psd,
        ):
            run_min = accp.tile([128, 512], f16, tag="run_min")
            run_min2 = accp.tile([128, 512], f32, tag="run_min2")
            nc.vector.memset(run_min[:], 60000.0)
            nc.vector.memset(run_min2[:], 3.0e38)

            # --- fp8 DoubleRow triangle sweep for the global min ---
            for b in range(NBLK):
                lt = stream.tile([128, KC, HS], f8, tag="lt")
                rt = stream.tile([128, KC, HS], f8, tag="rt")
                nc.sync.dma_start(out=lt[:], in_=lhs[b])
                nc.sync.dma_start(out=rt[:], in_=rhs[b])
                for m in range(MT):
                    for n in range(NT):
                        acc = ps.tile([128, 512], f32, tag="acc")
                        for k2 in range(2):
                            nc.tensor.matmul(
                                acc[:],
                                lt[:, 2 * k2 : 2 * k2 + 2, m * 128 : (m + 1) * 128],
                                rt[:, 2 * k2 : 2 * k2 + 2, n * 512 : (n + 1) * 512],
                                start=(k2 == 0), stop=(k2 == 1),
                                perf_mode=DR,
                            )
                        if (m * NT + n) % 4 != 3:
                            cp = small.tile([128, 512], f16, tag="cp")
                            nc.scalar.copy(cp[:], acc[:])
                            nc.vector.tensor_tensor(
                                run_min[:], run_min[:], cp[:], mybir.AluOpType.min
                            )
                        else:
                            nc.vector.tensor_tensor(
                                run_min2[:], run_min2[:], acc[:], mybir.AluOpType.min
                            )

            # --- exact-bf16 diagonal graph blocks (the gathered values) ---
            dgt = accp.tile([128, KC, GPC * G], bf16, tag="dgt")
            nc.sync.dma_start(out=dgt[:], in_=dg_in[:])
            for g in range(GPC):
                dacc = psd.tile([128, G], f32, tag="dacc")
                for k in range(KC):
                    nc.tensor.matmul(
                        dacc[:],
                        dgt[:, k, g * G : (g + 1) * G],
                        dgt[:, k, g * G : (g + 1) * G],
                        start=(k == 0), stop=(k == KC - 1),
                    )
                dcp = small.tile([128, G], f32, tag="dcp")
                nc.scalar.copy(dcp[:], dacc[:])
                nc.sync.dma_start(out=diag_out[g], in_=dcp[:])

            mm = small.tile([128, 2], f32, tag="mm")
            nc.vector.tensor_reduce(
                mm[:, 0:1], run_min[:], mybir.AxisListType.X, mybir.AluOpType.min
            )
            nc.vector.tensor_reduce(
                mm[:, 1:2], run_min2[:], mybir.AxisListType.X, mybir.AluOpType.min
            )
            nc.sync.dma_start(out=mins[:], in_=mm[:])

    nc.finalize()
    return nc


def _core_items(c: int):
    rows = [c, NHS - 1 - c]
    items = [(i, j) for i in rows for j in range(i, NHS)]
    assert len(items) == NBLK
    return items


def kernel(embeddings, row_idx, col_idx):
    global LAST_RESULTS
    from concourse.bass_utils import run_bass_kernel_spmd

    emb = np.asarray(embeddings, dtype=np.float32)
    row_idx = np.asarray(row_idx)
    col_idx = np.asarray(col_idx)

    if "nc" not in _CACHED:
        _CACHED["nc"] = _build_program()
    nc = _CACHED["nc"]

    eT = np.ascontiguousarray(emb.T)                       # [512, 16384] fp32
    e8 = eT.astype(ml_dtypes.float8_e4m3)                  # e4m3, RTNE
    # [slab, partition, d1, col]: per-partition-contiguous 4KB lines
    p8 = np.ascontiguousarray(
        e8.reshape(KC, 128, NHS, HS).transpose(2, 1, 0, 3)
    )
    e16 = eT.astype(ml_dtypes.bfloat16).reshape(KC, 128, NHS, HS)

    in_maps = []
    for c in range(NCORES):
        items = _core_items(c)
        lhs = p8[[i for i, _ in items]]
        rhs = p8[[j for _, j in items]]
        dg = np.ascontiguousarray(
            e16[:, :, [c, NHS - 1 - c], :].transpose(1, 0, 2, 3).reshape(
                128, KC, GPC * G
            )
        )
        in_maps.append({"lhs": lhs, "rhs": rhs, "dg": dg})

    import os
    res = run_bass_kernel_spmd(nc, in_maps, list(range(NCORES)))
    LAST_RESULTS = res

    m = min(
        min(r["mins"][:, 0].min(), r["mins"][:, 1].min()) for r in res.results
    )

    blocks = np.empty((N_GRAPHS, G, G), np.float32)
    gph = HS // G  # graphs per half-slab = 8
    for c in range(NCORES):
        rows = [c, NHS - 1 - c]
        gids = [i * gph + k for i in rows for k in range(gph)]
        for idx, g in enumerate(gids):
            blocks[g] = res.results[c]["diag_out"][idx]

    M = np.einsum("gii->gi", blocks).max()                 # global max (Cauchy-Schwarz)

    norm = (blocks - m) / (M - m + EPS)
    r = row_idx.astype(np.int64)
    cc = col_idx.astype(np.int64)
    out = norm[r >> 7, r & 127, cc & 127].astype(np.float32)
    return out
